# revision 1
# baseline (speedup 1.0000x reference)
"""Multi-head attention (B=2, L=2048, dim=1024, 16 heads) on 8 Trainium2 cores.

Sharding: 8 cores = 2 (batch) x 4 (head groups of 4 heads). Each core runs an
identical Bass program on its own slice (SPMD, no collectives); the host sums
the 4 per-head-group partial projection outputs per batch and adds the bias.

Per-core dataflow (bf16 matmul operands, fp32 PSUM accumulation):
  xT [1024, 2048]  (x[b] transposed, channel-major, bf16)
  V token-major [128 tok, 4 heads, 64+1] (ones column fused for the softmax
    denominator), qT/kT feature-major [128 (2 heads x 64d), 2048]
  ST[k, q] = kT.T @ qT    (K=64 contraction, head pairs row-packed in the PE;
                           the two matmuls run concurrently via row tiling)
  PT = exp(ST / 8)        head A: ScalarE table exp (PSUM -> SBUF bf16)
                          head B: DVE Schraudolph exp (bits = S*A + B written
                          as int16, bitcast to bf16; ~3% relative, cancels
                          in the softmax ratio)
  OT[d, q] += V.T @ PT    (M=65: row 64 accumulates the softmax denominator)
  normalization: per-head reciprocal_approx_fast on the denominator row,
    DMA-broadcast, head A multiply on GpSimd (SBUF copy), head B multiply on
    DVE directly from PSUM
  out[tok, c] = OT_norm.T @ wpT  -> PSUM, DMA'd straight to DRAM (fp32)

The exp is split Scalar/DVE because the attention steady state is otherwise
ScalarE-bound (1.08us per [128,1024] exp vs ~650ns of PE work per k-block).
"""

import os
import numpy as np

B, L, C = 2, 2048, 1024
H, D = 16, 64
HL = 4            # heads per core (local)
PAIRS = 2         # head pairs per core
CT = C // 128     # 8 contraction tiles for the projections
TOK = L // 128    # 16 key-token tiles
QW = 512          # query tile width
QS = L // QW      # 4 query tiles
NCORES = 8

# Schraudolph exp constants (bf16 bit pattern): bits = S_raw * EXPA + EXPB
import math
EXPA = 0.125 * math.log2(math.e) * 128.0
EXPB = 128.0 * (127.0 - 0.043035)
# k-blocks whose exp runs on the DVE via Schraudolph (full [128, 2*QW] tile
# in one op); the rest are exact table-exp on ScalarE. Alternating blocks
# balances the two engines at ~50% approx share.
DVE_KB = frozenset(range(1, TOK, 2))
KEEPWARM = True

_cache = {}


def _build_nc():
    import concourse.bass as bass
    import concourse.mybir as mybir
    import concourse.tile as tile
    from concourse import bacc

    F32 = mybir.dt.float32
    BF16 = mybir.dt.bfloat16
    I16 = mybir.dt.int16
    EXP = mybir.ActivationFunctionType.Exp
    MUL = mybir.AluOpType.mult
    ADD = mybir.AluOpType.add

    nc = bacc.Bacc("TRN2", target_bir_lowering=False, debug=False,
                   num_devices=NCORES)

    F16 = mybir.dt.float16
    xT = nc.declare_dram_parameter("xT", [C, L], BF16, isOutput=False)
    wT = nc.declare_dram_parameter("wT", [C, 3 * HL * D], BF16, isOutput=False)
    wpT = nc.declare_dram_parameter("wpT", [HL * D, C], BF16, isOutput=False)
    out = nc.declare_dram_parameter("out", [L, C], F16, isOutput=True)

    with tile.TileContext(nc) as tc:
        from contextlib import ExitStack
        with ExitStack() as ctx:
            qkpool = ctx.enter_context(tc.tile_pool(name="qk", bufs=1))
            vpool = ctx.enter_context(tc.tile_pool(name="v", bufs=1))
            wppool = ctx.enter_context(tc.tile_pool(name="wp", bufs=1))
            psS = ctx.enter_context(tc.tile_pool(name="psS", bufs=2, space="PSUM"))
            phase1 = ExitStack()
            xpool = phase1.enter_context(tc.tile_pool(name="x", bufs=1))
            wpool = phase1.enter_context(tc.tile_pool(name="w", bufs=1))
            psA = phase1.enter_context(tc.tile_pool(name="psA", bufs=2, space="PSUM"))

            # ---- input loads. Ordered so compute can start after ~1/4 of
            # the bytes land: the V-projection weights for every channel
            # tile first, then x in 512-column blocks across all channel
            # tiles (each V token tile / QK chunk needs one column block of
            # ALL channel tiles, not all columns of one).
            x_t = [xpool.tile([128, L], BF16, name=f"x{i}", tag=f"x{i}")
                   for i in range(CT)]
            w_t = [wpool.tile([128, 3 * HL * D], BF16, name=f"w{i}", tag=f"w{i}")
                   for i in range(CT)]
            # Issuing a DMA descriptor costs its queue ~0.77us, so the loads
            # are spread round-robin over the four idle engine queues and
            # kept coarse (issue time, not HBM bandwidth, is the limiter).
            qs_engines = [nc.sync, nc.scalar, nc.gpsimd]
            _dma_rr = [0]

            def dma_rr(out_ap, in_ap):
                eng = qs_engines[_dma_rr[0] % len(qs_engines)]
                _dma_rr[0] += 1
                eng.dma_start(out=out_ap, in_=in_ap)

            for i in range(CT):   # V weights (cols 512:768) for all c first
                dma_rr(w_t[i][:, 2 * HL * D:3 * HL * D],
                       wT[128 * i:128 * (i + 1), 2 * HL * D:3 * HL * D])
            for i in range(CT):   # x first query block
                dma_rr(x_t[i][:, 0:QW], xT[128 * i:128 * (i + 1), 0:QW])
            for i in range(CT):   # Q/K weights
                dma_rr(w_t[i][:, 0:2 * HL * D],
                       wT[128 * i:128 * (i + 1), 0:2 * HL * D])
            for i in range(CT):
                dma_rr(x_t[i][:, QW:2 * QW],
                       xT[128 * i:128 * (i + 1), QW:2 * QW])
            for i in range(CT):
                dma_rr(x_t[i][:, 2 * QW:4 * QW],
                       xT[128 * i:128 * (i + 1), 2 * QW:4 * QW])
            wp_t = []
            for p in range(PAIRS):
                t = wppool.tile([128, C], BF16, name=f"wp{p}", tag=f"wp{p}")
                dma_rr(t, wpT[2 * D * p:2 * D * (p + 1), :])
                wp_t.append(t)

            # ---- V token-major: v[t] = [128 tok, HL, D+1] (ones col fused) --
            ones_s = vpool.tile([128, HL, 1], F32, name="ones_s", tag="ones_s")
            nc.vector.memset(ones_s, 1.0)
            v_t = [vpool.tile([128, HL, D + 1], BF16, name=f"v{t}", tag=f"v{t}")
                   for t in range(TOK)]

            def vchunk(t):
                ps = psA.tile([128, HL * D], F32, name="psv", tag="ps")
                for c in range(CT):
                    nc.tensor.matmul(
                        ps,
                        lhsT=x_t[c][:, 128 * t:128 * (t + 1)],
                        rhs=w_t[c][:, 2 * HL * D:3 * HL * D],
                        start=(c == 0), stop=(c == CT - 1),
                    )
                vt = v_t[t]
                nc.vector.tensor_copy(out=vt[:, :, D:D + 1], in_=ones_s)
                nc.vector.tensor_copy(
                    out=vt[:, :, 0:D],
                    in_=ps.rearrange("p (h d) -> p h d", h=HL),
                )

            # ---- Q/K feature-major per pair: [128 (2h x 64d), L] ------------
            qk_t = {}
            for p in range(PAIRS):
                for nm in ("q", "k"):
                    qk_t[(nm, p)] = qkpool.tile(
                        [128, L], BF16, name=f"{nm}{p}", tag=f"{nm}{p}")

            def qkchunk(nm, p, ns):
                j = 0 if nm == "q" else 1
                ps = psA.tile([128, QW], F32, name="psqk", tag="ps")
                for c in range(CT):
                    nc.tensor.matmul(
                        ps,
                        lhsT=w_t[c][:, j * HL * D + 128 * p:
                                    j * HL * D + 128 * (p + 1)],
                        rhs=x_t[c][:, QW * ns:QW * (ns + 1)],
                        start=(c == 0), stop=(c == CT - 1),
                    )
                nc.vector.tensor_copy(
                    out=qk_t[(nm, p)][:, QW * ns:QW * (ns + 1)], in_=ps)

            # emission grouped by which x column block each matmul needs, so
            # an x-DMA wait never blocks work whose data already landed
            for t in range(4):
                vchunk(t)
            qkchunk("k", 0, 0), qkchunk("k", 1, 0)
            for t in range(4, 8):
                vchunk(t)
            qkchunk("k", 0, 1), qkchunk("k", 1, 1)
            qkchunk("q", 0, 0), qkchunk("q", 1, 0)
            for t in range(8, 12):
                vchunk(t)
            qkchunk("k", 0, 2), qkchunk("k", 1, 2)
            qkchunk("q", 0, 1), qkchunk("q", 1, 1)
            for t in range(12, 16):
                vchunk(t)
            qkchunk("k", 0, 3), qkchunk("k", 1, 3)
            for ns in (2, 3):
                qkchunk("q", 0, ns), qkchunk("q", 1, ns)

            phase1.close()
            # ---- phase 2 pools (reuse the x/w SBUF + psA PSUM space) --------
            psO = ctx.enter_context(tc.tile_pool(name="psO", bufs=4, space="PSUM"))
            otpool = ctx.enter_context(tc.tile_pool(name="ot", bufs=1))
            ptpool = ctx.enter_context(tc.tile_pool(name="pt", bufs=3))
            rpool = ctx.enter_context(tc.tile_pool(name="r", bufs=2))
            rpool2 = ctx.enter_context(tc.tile_pool(name="r2", bufs=2))
            obpool = ctx.enter_context(tc.tile_pool(name="ob", bufs=4))

            def proj_chunk(qs, last=False):
                # PSUM -> fp16 SBUF staging split across ScalarE/DVE, then out.
                # The last chunk's DMAs are split in half and spread over two
                # issue queues so the end-of-kernel drain is shorter.
                for t in range(QW // 128 * qs, QW // 128 * (qs + 1)):
                    ob = obpool.tile([128, C], F16, name="ob", tag="ob")
                    for nh in range(C // QW):
                        ps = psO.tile([128, QW], F32, name="psp", tag="ot")
                        for p2 in range(PAIRS):
                            nc.tensor.matmul(
                                ps,
                                lhsT=ot_sb[p2][qs][:, 128 * (t % (QW // 128)):
                                                   128 * (t % (QW // 128) + 1)],
                                rhs=wp_t[p2][:, QW * nh:QW * (nh + 1)],
                                start=(p2 == 0), stop=(p2 == PAIRS - 1),
                            )
                        dst = ob[:, QW * nh:QW * (nh + 1)]
                        if nh == 0:
                            nc.scalar.copy(out=dst, in_=ps)
                        else:
                            nc.vector.tensor_copy(out=dst, in_=ps)
                        orow = out[128 * t:128 * (t + 1), :]
                        if last:
                            h = QW // 2
                            c0 = QW * nh
                            nc.sync.dma_start(
                                out=orow[:, c0:c0 + h],
                                in_=dst[:, 0:h])
                            nc.gpsimd.dma_start(
                                out=orow[:, c0 + h:c0 + QW],
                                in_=dst[:, h:QW])
                        else:
                            nc.sync.dma_start(
                                out=orow[:, QW * nh:QW * (nh + 1)], in_=dst)

            # ---- attention --------------------------------------------------
            # One [64, QW] normalized output tile per local head per query
            # chunk; per-chunk tiles keep the projection's reads free of
            # false dependencies on the next chunk's normalization writes
            # (DMA writes are dependency-tracked at tile granularity).
            # O matmuls are M=65 (64 V columns + ones column -> denominator
            # in psum row 64); exactly one accumulation group per PSUM bank.
            ot_sb = [[otpool.tile([128, QW], BF16, name=f"otp{p}q{q}",
                                  tag=f"otp{p}q{q}")
                      for q in range(QS)] for p in range(PAIRS)]
            for qs in range(QS):
                for p in range(PAIRS):
                    kT = qk_t[("k", p)]
                    qT = qk_t[("q", p)]
                    last_pair = (qs == QS - 1 and p == PAIRS - 1)
                    ot_a = psO.tile([65, QW], F32, name="ot_a", tag="ot")
                    ot_b = psO.tile([65, QW], F32, name="ot_b", tag="ot")
                    # The engine queues are strict FIFO: a PV matmul waiting
                    # on its exp would block ready score matmuls queued
                    # behind it. Emitting each PV three k-blocks behind its
                    # exp gives the exp ~3 iterations of latency slack —
                    # enough to also absorb the pair-boundary delay where
                    # the first exps queue behind the previous pair's PSUM
                    # evacuation copies.
                    pend = {}
                    for kb in range(TOK + 3):
                        if kb < TOK:
                            st = psS.tile([128, 2 * QW], F32,
                                          name="st", tag="st")
                            # scores for both heads (row-packed K=64, the
                            # two matmuls run concurrently via row tiling)
                            nc.tensor.matmul(
                                st[:, 0:QW],
                                lhsT=kT[0:64, 128 * kb:128 * (kb + 1)],
                                rhs=qT[0:64, QW * qs:QW * (qs + 1)],
                                start=True, stop=True,
                            )
                            nc.tensor.matmul(
                                st[:, QW:2 * QW],
                                lhsT=kT[64:128, 128 * kb:128 * (kb + 1)],
                                rhs=qT[64:128, QW * qs:QW * (qs + 1)],
                                start=True, stop=True,
                            )
                            # one full-tile exp per k-block, alternating
                            # engines: exact table exp on ScalarE /
                            # Schraudolph bit-trick exp on the DVE. Tiles
                            # are written in their native dtype and only
                            # READ through bitcast (a bitcast write breaks
                            # dependency tracking).
                            if kb in DVE_KB:
                                pti = ptpool.tile([128, 2 * QW], I16,
                                                  name="pti", tag="pti")
                                nc.vector.tensor_scalar(
                                    out=pti, in0=st,
                                    scalar1=EXPA, scalar2=EXPB,
                                    op0=MUL, op1=ADD)
                                pend[kb] = pti.bitcast(BF16)
                            else:
                                pt = ptpool.tile([128, 2 * QW], BF16,
                                                 name="pt", tag="pt")
                                nc.scalar.activation(
                                    out=pt, in_=st, func=EXP, scale=0.125)
                                pend[kb] = pt
                        if kb >= 3:
                            kv = kb - 3
                            pt = pend.pop(kv)
                            # O accumulation (64 V cols + ones col per head)
                            nc.tensor.matmul(
                                ot_a,
                                lhsT=v_t[kv][:, 2 * p, :],
                                rhs=pt[:, 0:QW],
                                start=(kv == 0), stop=(kv == TOK - 1),
                            )
                            nc.tensor.matmul(
                                ot_b,
                                lhsT=v_t[kv][:, 2 * p + 1, :],
                                rhs=pt[:, QW:2 * QW],
                                start=(kv == 0), stop=(kv == TOK - 1),
                            )
                    # Normalization. Both heads are copied out of PSUM by
                    # ScalarE right away (frees the O banks for the next
                    # pair / projection). The denominator rows (row 64) move
                    # to partition 0 via SBUF-SBUF DMAs because
                    # reciprocal_approx_fast only works at base partition 0;
                    # one wide recip, one broadcast DMA, multiplies on
                    # GpSimd (DVE for the last pair: lower latency there).
                    den0 = rpool.tile([1, 2 * QW], F32, name="den0", tag="den0")
                    oc_a = rpool2.tile([65, QW], F32, name="oc_a", tag="oc_a")
                    oc_b = rpool2.tile([65, QW], F32, name="oc_b", tag="oc_b")
                    nc.scalar.copy(out=oc_a, in_=ot_a)
                    nc.scalar.copy(out=oc_b, in_=ot_b)
                    nc.gpsimd.dma_start(out=den0[0:1, 0:QW],
                                        in_=oc_a[64:65, :])
                    nc.sync.dma_start(out=den0[0:1, QW:2 * QW],
                                      in_=oc_b[64:65, :])
                    rsb = rpool.tile([1, 2 * QW], F32, name="rsb", tag="rsb")
                    nc.vector.reciprocal_approx_fast(out=rsb, in_=den0)
                    # native GpSimd broadcast: the DMA version of this write
                    # (256KB fan-out) costs >10us of DMA-queue time per pair
                    rbc = rpool.tile([64, 2 * QW], F32, name="rbc", tag="rbc")
                    nc.gpsimd.partition_broadcast(rbc, rsb[0:1, :])
                    if last_pair and KEEPWARM:
                        # tiny matmuls chained on the norm-chain tensors keep
                        # the PE activity monitor from re-throttling the
                        # clock (idle >3.4us -> 1.2GHz) before the final
                        # projection; each costs ~30ns
                        kw = psS.tile([128, 2 * QW], F32, name="kw", tag="st")
                        nc.tensor.matmul(kw[0:64, 0:64], lhsT=den0[0:1, 0:64],
                                         rhs=den0[0:1, QW:QW + 64],
                                         start=True, stop=True)
                    # the odd head is written straight into partitions 64-127
                    # by the DVE (64-wide quadrant-pair-aligned ops can cross
                    # halves), removing the cross-partition staging DMA that
                    # used to gate the projection at every chunk boundary
                    nc.vector.tensor_mul(
                        out=ot_sb[p][qs][64:128, :],
                        in0=oc_b[0:64, :], in1=rbc[:, QW:2 * QW])
                    mul_eng = nc.vector if last_pair else nc.gpsimd
                    mul_eng.tensor_mul(
                        out=ot_sb[p][qs][0:64, :],
                        in0=oc_a[0:64, :], in1=rbc[:, 0:QW])
                    if last_pair and KEEPWARM:
                        kw2 = psS.tile([128, 2 * QW], F32, name="kw2", tag="st")
                        nc.tensor.matmul(kw2[0:64, 0:64], lhsT=rbc[:, 0:64],
                                         rhs=rbc[:, QW:QW + 64],
                                         start=True, stop=True)

                # next query chunk's output projection (inputs long since
                # ready -> no PE stall)
                if qs > 0:
                    proj_chunk(qs - 1)
            proj_chunk(QS - 1, last=True)

    nc.compile()
    return nc


def _get_nc():
    if "nc" not in _cache:
        _cache["nc"] = _build_nc()
    return _cache["nc"]


def kernel(x, w_qkv, w_proj, b_proj):
    import ml_dtypes
    from concourse.bass_utils import run_bass_kernel_spmd

    x = np.asarray(x, dtype=np.float32)
    w_qkv = np.asarray(w_qkv, dtype=np.float32)
    w_proj = np.asarray(w_proj, dtype=np.float32)
    b_proj = np.asarray(b_proj, dtype=np.float32)

    nc = _get_nc()
    in_maps = []
    for core in range(NCORES):
        b, g = divmod(core, 4)
        rows = np.concatenate([
            np.arange(C * j + HL * D * g, C * j + HL * D * (g + 1))
            for j in range(3)
        ])
        in_maps.append({
            "xT": np.ascontiguousarray(x[b].T).astype(ml_dtypes.bfloat16),
            "wT": np.ascontiguousarray(w_qkv[rows].T).astype(ml_dtypes.bfloat16),
            "wpT": np.ascontiguousarray(
                w_proj[:, HL * D * g:HL * D * (g + 1)].T).astype(ml_dtypes.bfloat16),
        })

    res = run_bass_kernel_spmd(
        nc, in_maps, list(range(NCORES)),
        trace=bool(os.environ.get("KERNEL_TRACE")),
    )
    _cache["last_results"] = res

    out = np.empty((B, L, C), dtype=np.float32)
    for b in range(B):
        acc = res.results[4 * b]["out"].astype(np.float32)
        for g in range(1, 4):
            acc = acc + res.results[4 * b + g]["out"]
        out[b] = acc + b_proj[None, :]
    return out

